# revision 14
# baseline (speedup 1.0000x reference)
"""ComplexMultiHeadAttention on 8 TRN2 NeuronCores (Bass/Tile).

Problem: B=4, S=1024, D_MODEL=1024, N_HEADS=16, D_HEAD=64, complex-valued
activations stored as a trailing dim of size 2 (real, imag).

    q = to_heads(complex_linear(queries, wq));  k, v likewise
    s_r + i*s_i = (q_r + i q_i)(k_r + i k_i)^T / sqrt(dh)
    a_r = softmax(s_r), a_i = softmax(s_i)      (independent softmaxes)
    o = complex_bmm(a, v);  out = complex_linear(concat_heads(o), wo)

Sharding: head-parallel. Core c owns heads {2c, 2c+1} = 128 contiguous dims
of the hidden axis. Each core computes Q/K/V projections for its 128 output
dims (weights row-sliced), runs attention for its 8 (batch, head) pairs, and
computes a partial O-projection (wo column-sliced on its 128 input dims)
over all 1024 output dims. The host sums the 8 partial outputs — no
on-device collectives.

Layout: tokens always on the FREE dim, features/keys on partitions, so
every matmul is a natural lhsT.T @ rhs with K=128 contraction:
  - inputs passed transposed: x^T [1024 d, 4096 t] (bf16; halves DMA),
    re-tiled on host so one (512-token half)'s 8 K-chunks are CONTIGUOUS:
    each projection unit loads ONE [128, 8*512] macro-tile per r/i stream
    (1 dma_start instead of 8 — dma_start issue costs ~0.6us each and
    dominated kernel warmup). Each 512-col block feeds TWO matmuls,
    keeping projections PE-bound, not DMA-bound.
  - projections produce psum [128 outdims, 512 t]; the complex parts are
    handled by accumulating with sign-folded weight copies (w_i, -w_i).
  - scores are computed TRANSPOSED (s^T [k, q]) from Qcat = [q_r; q_i],
    Kcat_r = [k_r; -k_i], Kcat_i = [k_i; k_r] (all [128, S] f32r) — one
    K=128 matmul per 128-key chunk, no accumulation.
  - softmax over k (= partitions) skips max-subtraction (scores are O(1)
    by construction, exp cannot overflow). exp writes bf16 u-tiles; the
    8 per-part u tiles are pairwise tree-summed on DVE (7 bf16 adds) and
    ONE ones[128,128]-matmul replicates the total Z across partitions,
    making the 1/Z scale an aligned tensor_mul.
  - V is PE-transposed into token-major packs VA=[v_r|v_i], VB=[v_i|v_r],
    so attn@V accumulates o_pack [o_r|o_i, q] in a single psum group.

Scheduling (the key to p-state): TRN2's PE runs ~2x slower unless it has
been continuously busy for ~3us, so every dependency bubble costs twice.
Projection work for batch b+1 is emitted through GENERATORS that yield
after each 2-matmul step; the attention inner loop pulls two steps per
key-chunk so ALL projection work hides inside attention groups (a bulk
drain at batch end measurably stalls the PE on staging backlog). The
prologue (batch 0) runs unit PAIRS round-robin on disjoint psum pools so
one unit's matmuls cover the other's staging bubbles. PSUM pools are
split by consumer engine so score matmuls never wait on banks drained by
slow queued DVE work: projps 2 (projection accumulators + V-transpose),
sps 2 (scores + Z), ojps 2 (O-projection, drained by Scalar copies; also
the prologue's second projection ring), ops 2 (AV accumulators).
Weights load as two packed tensors (q first) so the first matmul starts
after ~1.3MB of DMA, not all of it.

Matmul dtype note (cost-model + HW verified): bf16 and f32r both run at
1 cycle/row for >=256-row moving operands, so dtype choice is about DMA
bytes and precision, not PE speed. Scores/Q/K stay f32r in SBUF (exp is
the error-sensitive consumer); x/weights/u/V/outputs are bf16.
"""

import os
import numpy as np
import ml_dtypes
from collections import deque
from contextlib import ExitStack

import concourse.bass as bass
import concourse.tile as tile
from concourse import bacc, mybir

F32 = mybir.dt.float32
F32R = mybir.dt.float32r
BF16 = mybir.dt.bfloat16
EXP = mybir.ActivationFunctionType.Exp

B, S, D, H, DH = 4, 1024, 1024, 16, 64
NCORES = 8
P = 128            # partitions / chunk size
TBLK = 512         # token block (matmul free dim)
DC = D // P        # 8 d-chunks
KC = S // P        # 8 key chunks per batch
HPC = H // NCORES  # 2 heads per core
NT = (B * S) // TBLK  # 8 token blocks

QPACKS = ("a0", "a1", "b0", "b1")
KVPACKS = tuple(f"k{s}" for s in QPACKS) + tuple(f"v{s}" for s in QPACKS) \
    + ("wo_r", "wo_i", "wo_in")

_CACHE = {}


def _build():
    nc = bacc.Bacc("TRN2", target_bir_lowering=False, debug=False,
                   num_devices=NCORES)

    x_ap = {}
    for t in ("q", "k", "v"):
        for part in ("r", "i"):
            # gt-major tiled layout: rows (gt*DC + dc)*P : +P, so one
            # 512-token half is a single contiguous [DC*P, TBLK] region
            x_ap[t + part] = nc.dram_tensor(
                f"x{t}_{part}", [NT * DC * P, TBLK],
                BF16, kind="ExternalInput").ap()
    wq_ap = nc.dram_tensor("w_qpack", [P, len(QPACKS) * D], BF16,
                           kind="ExternalInput").ap()
    wkv_ap = nc.dram_tensor("w_kvpack", [P, len(KVPACKS) * D], BF16,
                            kind="ExternalInput").ap()
    ident_ap = nc.dram_tensor("ident", [P, P], BF16, kind="ExternalInput").ap()
    ones_ap = nc.dram_tensor("onesin", [P, P], BF16, kind="ExternalInput").ap()
    # tiled-contiguous outputs: row block (gt*DC + mc)*P
    po_r = nc.dram_tensor("po_r", [NT * DC * P, TBLK], BF16,
                          kind="ExternalOutput").ap()
    po_i = nc.dram_tensor("po_i", [NT * DC * P, TBLK], BF16,
                          kind="ExternalOutput").ap()

    with tile.TileContext(nc) as tc, ExitStack() as ctx:
        wpool = ctx.enter_context(tc.tile_pool(name="w", bufs=1))
        xpool = ctx.enter_context(tc.tile_pool(name="x", bufs=4))
        qkpool = ctx.enter_context(tc.tile_pool(name="qk", bufs=2))
        vpool = ctx.enter_context(tc.tile_pool(name="v", bufs=2))
        opool = ctx.enter_context(tc.tile_pool(name="ost", bufs=2))
        upool = ctx.enter_context(tc.tile_pool(name="u", bufs=8))
        uspool = ctx.enter_context(tc.tile_pool(name="us", bufs=8))
        zpool = ctx.enter_context(tc.tile_pool(name="z", bufs=2))
        tmppool = ctx.enter_context(tc.tile_pool(name="tmp", bufs=4))
        popool = ctx.enter_context(tc.tile_pool(name="po", bufs=4))
        vstpool = ctx.enter_context(tc.tile_pool(name="vst", bufs=2))
        # PSUM: 8 banks, split by consumer so engines don't cross-block.
        projps = ctx.enter_context(tc.tile_pool(name="pp", bufs=2, space="PSUM"))
        sps = ctx.enter_context(tc.tile_pool(name="sp", bufs=2, space="PSUM"))
        ojps = ctx.enter_context(tc.tile_pool(name="oj", bufs=2, space="PSUM"))
        ops_pool = ctx.enter_context(tc.tile_pool(name="op", bufs=1, space="PSUM"))

        # packed weights: (tile, base column) per logical pack
        wq_all = wpool.tile([P, len(QPACKS) * D], BF16, tag="wq", name="wq")
        wkv_all = wpool.tile([P, len(KVPACKS) * D], BF16, tag="wkv",
                             name="wkv")
        wcol = {}
        for i, sfx in enumerate(QPACKS):
            wcol["q" + sfx] = (wq_all, i * D)
        for i, key in enumerate(KVPACKS):
            wcol[key] = (wkv_all, i * D)

        # per-batch staged tiles (created lazily, rotated by pool bufs=2)
        stage = {}

        def get_stage(b):
            if b not in stage:
                stage[b] = {
                    "qcat": [qkpool.tile([P, S], F32R, tag=f"qcat{h}",
                                         name=f"qcat{h}") for h in range(HPC)],
                    "kcr": [qkpool.tile([P, S], F32R, tag=f"kcr{h}",
                                        name=f"kcr{h}") for h in range(HPC)],
                    "kci": [qkpool.tile([P, S], F32R, tag=f"kci{h}",
                                        name=f"kci{h}") for h in range(HPC)],
                    "va": [vpool.tile([P, S], BF16, tag=f"va{h}",
                                      name=f"va{h}") for h in range(HPC)],
                    "vb": [vpool.tile([P, S], BF16, tag=f"vb{h}",
                                      name=f"vb{h}") for h in range(HPC)],
                    "o": {p: opool.tile([P, S], BF16, tag=f"ost{p}",
                                        name=f"ost{p}") for p in ("r", "i")},
                }
            return stage[b]

        def wsl(t, sfx, dc):
            tile_, base = wcol[(t + sfx) if t != "o" else sfx]
            return tile_[:, base + dc * P: base + (dc + 1) * P]

        def xmacro(t, part, gt):
            """Prefetch a full half's 8 chunks into one [128, 8*512]
            macro-tile (8 contiguous per-chunk DMAs, issued a unit ahead
            of their matmuls by the fill driver)."""
            xt = xpool.tile([P, DC * TBLK], BF16, tag="xt", name="xt")
            for dc in range(DC):
                r0 = (gt * DC + dc) * P
                nc.sync.dma_start(xt[:, dc * TBLK:(dc + 1) * TBLK],
                                  x_ap[t + part][r0:r0 + P, :])
            return xt

        def proj_unit_gen(b, t, half, pool, ptag, xrt, xit):
            """Projection of one (tensor, 512-token half): 32 matmuls fed
            by two PRE-ISSUED macro-tiles (their DMAs start a unit ahead
            so no matmul heads the in-order PE queue waiting on a 1MB
            transfer). Yields after every chunk so the driver can
            interleave it into PE bubbles."""
            st = get_stage(b)
            psr = pool.tile([P, TBLK], F32, tag=ptag, name="projps")
            psi = pool.tile([P, TBLK], F32, tag=ptag, name="projps")
            for dc in range(DC):
                xs_ = slice(dc * TBLK, (dc + 1) * TBLK)
                nc.tensor.matmul(psr[:], wsl(t, "a0", dc), xrt[:, xs_],
                                 start=(dc == 0), stop=False)
                nc.tensor.matmul(psi[:], wsl(t, "a1", dc), xrt[:, xs_],
                                 start=(dc == 0), stop=False)
                yield
            for dc in range(DC):
                xs_ = slice(dc * TBLK, (dc + 1) * TBLK)
                nc.tensor.matmul(psr[:], wsl(t, "b0", dc), xit[:, xs_],
                                 start=False, stop=(dc == DC - 1))
                nc.tensor.matmul(psi[:], wsl(t, "b1", dc), xit[:, xs_],
                                 start=False, stop=(dc == DC - 1))
                yield
            hs = slice(half * TBLK, (half + 1) * TBLK)
            if t == "q":
                # psX = [q_r(h); q_i(h)] = Qcat directly
                for h, psx in ((0, psr), (1, psi)):
                    nc.vector.tensor_copy(st["qcat"][h][:, hs], psx[:])
            elif t == "k":
                # psX = [k_r(h); -k_i(h)] = Kcat_r directly;
                # Kcat_i = [k_i; k_r] via one negate + one copy
                for h, psx in ((0, psr), (1, psi)):
                    nc.vector.tensor_copy(st["kcr"][h][:, hs], psx[:])
                    nc.vector.tensor_scalar_mul(st["kci"][h][0:DH, hs],
                                                psx[DH:P, :], -1.0)
                    nc.vector.tensor_copy(st["kci"][h][DH:P, hs],
                                          psx[0:DH, :])
            else:
                # psr = [v_r(h0); v_i(h0)], psi = [v_r(h1); v_i(h1)]
                for h, psx in ((0, psr), (1, psi)):
                    vst = vstpool.tile([P, TBLK], BF16, tag="vst", name="vst")
                    nc.vector.tensor_copy(vst[:], psx[:])
                    ptb = pool.tile([P, TBLK], BF16, tag=ptag, name="ptb")
                    for blk in range(4):
                        bs = slice(blk * P, (blk + 1) * P)
                        nc.tensor.transpose(ptb[:, bs], vst[:, bs], ident[:])
                    # ptb cols per blk: [v_r(h) 64 | v_i(h) 64]
                    base = half * TBLK
                    nc.vector.tensor_copy(st["va"][h][:, base:base + TBLK],
                                          ptb[:])
                    vbv = st["vb"][h][:, base:base + TBLK].rearrange(
                        "p (k c) -> p k c", c=P)
                    ptv = ptb[:].rearrange("p (k c) -> p k c", c=P)
                    nc.vector.tensor_copy(vbv[:, :, 0:DH], ptv[:, :, DH:P])
                    nc.vector.tensor_copy(vbv[:, :, DH:P], ptv[:, :, 0:DH])
                    yield

        # filler driver: background projection work pulled into PE bubbles.
        # x macro-DMAs are issued one unit AHEAD of matmul emission.
        fill_state = {"gen": None, "dma_q": deque(), "ready": deque()}

        def prefetch_unit():
            if fill_state["dma_q"]:
                b, t, half = fill_state["dma_q"].popleft()
                gt = 2 * b + half
                fill_state["ready"].append(
                    (b, t, half, xmacro(t, "r", gt), xmacro(t, "i", gt)))

        def fill(n):
            for _ in range(n):
                while True:
                    if fill_state["gen"] is None:
                        if not fill_state["ready"]:
                            if not fill_state["dma_q"]:
                                return
                            prefetch_unit()
                        b, t, half, xrt, xit = fill_state["ready"].popleft()
                        prefetch_unit()
                        fill_state["gen"] = proj_unit_gen(
                            b, t, half, projps, "projps", xrt, xit)
                    try:
                        next(fill_state["gen"])
                        break
                    except StopIteration:
                        fill_state["gen"] = None

        def fill_drain():
            fill(1 << 30)

        def emit_attn_group(b, h, qb):
            """One (head, 512-query block): 32 score/AV matmuls + 2 Z,
            pulling filler work into every exp-gated bubble."""
            st = get_stage(b)
            qs = slice(qb * TBLK, (qb + 1) * TBLK)
            ota = ops_pool.tile([P, TBLK], F32, tag="ota", name="ota")
            otb = ops_pool.tile([P, TBLK], F32, tag="otb", name="otb")
            acc = {"r": [], "i": []}  # pairwise tree partials

            def tree_push(part, t_new):
                lst = acc[part]
                lst.append((0, t_new))
                while len(lst) >= 2 and lst[-1][0] == lst[-2][0]:
                    r1, a = lst.pop()
                    _, bt = lst.pop()
                    s = uspool.tile([P, TBLK], BF16, tag=f"us{part}",
                                    name=f"us{part}")
                    nc.vector.tensor_add(s[:], a[:], bt[:])
                    lst.append((r1 + 1, s))

            for kc in range(KC):
                ks = slice(kc * P, (kc + 1) * P)
                first, last = kc == 0, kc == KC - 1
                str_ = sps.tile([P, TBLK], F32, tag="sps", name="sps")
                nc.tensor.matmul(str_[:], st["kcr"][h][:, ks],
                                 st["qcat"][h][:, qs], start=True, stop=True)
                ur = upool.tile([P, TBLK], BF16, tag="u", name="u")
                nc.scalar.activation(ur[:], str_[:], EXP)
                sti = sps.tile([P, TBLK], F32, tag="sps", name="sps")
                nc.tensor.matmul(sti[:], st["kci"][h][:, ks],
                                 st["qcat"][h][:, qs], start=True, stop=True)
                ui = upool.tile([P, TBLK], BF16, tag="u", name="u")
                nc.scalar.activation(ui[:], sti[:], EXP)
                nc.tensor.matmul(ota[:], st["va"][h][:, ks], ur[:],
                                 start=first, stop=last)
                nc.tensor.matmul(otb[:], st["vb"][h][:, ks], ui[:],
                                 start=first, stop=last)
                tree_push("r", ur)
                tree_push("i", ui)
                fill(2)
            usum = {}
            for part in ("r", "i"):
                lst = acc[part]
                while len(lst) >= 2:  # KC is a power of 2, but be safe
                    _, a = lst.pop()
                    _, bt = lst.pop()
                    s = uspool.tile([P, TBLK], BF16, tag=f"us{part}",
                                    name=f"us{part}")
                    nc.vector.tensor_add(s[:], a[:], bt[:])
                    lst.append((99, s))
                usum[part] = lst[0][1]
            # Z replicated across partitions via one ones-matmul per part;
            # each AV term gets its OWN denominator (independent softmaxes).
            # Z psums come from the sps ring: no shared-bank serialization.
            zinv = {}
            for part in ("r", "i"):
                zps = sps.tile([P, TBLK], F32, tag="sps", name="zsum")
                nc.tensor.matmul(zps[:], ones[:], usum[part][:],
                                 start=True, stop=True)
                fill(1)
                zinv[part] = zpool.tile([P, TBLK], F32, tag="zinv",
                                        name=f"zinv{part}")
                nc.vector.reciprocal_approx_fast(zinv[part][:], zps[:])
            tmpa = tmppool.tile([P, TBLK], F32, tag="tmp", name="tmpa")
            nc.vector.tensor_mul(tmpa[:], ota[:], zinv["r"][:])
            tmpb = tmppool.tile([P, TBLK], F32, tag="tmp", name="tmpb")
            nc.vector.tensor_mul(tmpb[:], otb[:], zinv["i"][:])
            dst = slice(DH * h, DH * (h + 1))
            nc.vector.tensor_sub(st["o"]["r"][dst, qs], tmpa[0:DH, :],
                                 tmpb[0:DH, :])
            nc.vector.tensor_add(st["o"]["i"][dst, qs], tmpa[DH:P, :],
                                 tmpb[DH:P, :])
            fill(4)

        def emit_oproj(b, half):
            """Partial O-projection for one 512-token half: 32 matmuls
            in a dedicated psum ring drained by Scalar copies."""
            st = get_stage(b)
            hs = slice(half * TBLK, (half + 1) * TBLK)
            gt = 2 * b + half
            for mc in range(DC):
                orow = (gt * DC + mc) * P
                pr = ojps.tile([P, TBLK], F32, tag="ojps", name="ojpr")
                nc.tensor.matmul(pr[:], wsl("o", "wo_r", mc),
                                 st["o"]["r"][:, hs], start=True, stop=False)
                nc.tensor.matmul(pr[:], wsl("o", "wo_in", mc),
                                 st["o"]["i"][:, hs], start=False, stop=True)
                sbr = popool.tile([P, TBLK], BF16, tag="po", name="po")
                nc.any.tensor_copy(sbr[:], pr[:])
                nc.sync.dma_start(po_r[orow:orow + P, :], sbr[:])
                pi = ojps.tile([P, TBLK], F32, tag="ojps", name="ojpi")
                nc.tensor.matmul(pi[:], wsl("o", "wo_i", mc),
                                 st["o"]["r"][:, hs], start=True, stop=False)
                nc.tensor.matmul(pi[:], wsl("o", "wo_r", mc),
                                 st["o"]["i"][:, hs], start=False, stop=True)
                sbi = popool.tile([P, TBLK], BF16, tag="po", name="po")
                nc.any.tensor_copy(sbi[:], pi[:])
                nc.sync.dma_start(po_i[orow:orow + P, :], sbi[:])
                fill(1)

        # ---- prologue: q-weights first so matmuls start early; batch-0
        # units run in PAIRS on disjoint psum pools so one unit's matmuls
        # cover the other's staging bubbles.
        nc.sync.dma_start(wq_all[:], wq_ap[:])
        ident = wpool.tile([P, P], BF16, tag="ident", name="ident")
        nc.sync.dma_start(ident[:], ident_ap[:])
        ones = wpool.tile([P, P], BF16, tag="ones", name="ones")
        nc.sync.dma_start(ones[:], ones_ap[:])

        def run_pair(ua, ub, second_dma=None):
            tiles = []
            for (b_, t_, half_) in (ua, ub):
                gt = 2 * b_ + half_
                tiles.append((xmacro(t_, "r", gt), xmacro(t_, "i", gt)))
            gens = [proj_unit_gen(*ua, projps, "projps", *tiles[0]),
                    proj_unit_gen(*ub, ojps, "ojps", *tiles[1])]
            if second_dma is not None:
                second_dma()
            while gens:
                for g in list(gens):
                    try:
                        next(g)
                    except StopIteration:
                        gens.remove(g)

        run_pair((0, "q", 0), (0, "q", 1),
                 lambda: nc.sync.dma_start(wkv_all[:], wkv_ap[:]))
        run_pair((0, "k", 0), (0, "k", 1))
        run_pair((0, "v", 0), (0, "v", 1))

        # ---- steady state: attention(b) with projection(b+1) pulled in
        # as fine-grained filler; oproj(b, half) as soon as its half of
        # o_stage completes. Unit order puts K/V (needed by the FIRST
        # group of b+1) ahead of q half 1 (needed only by the third).
        for b in range(B):
            if b + 1 < B:
                fill_state["dma_q"] = deque(
                    [(b + 1, "q", 0), (b + 1, "k", 0), (b + 1, "k", 1),
                     (b + 1, "v", 0), (b + 1, "v", 1), (b + 1, "q", 1)])
                prefetch_unit()
            emit_attn_group(b, 0, 0)
            emit_attn_group(b, 1, 0)
            emit_oproj(b, 0)
            emit_attn_group(b, 0, 1)
            emit_attn_group(b, 1, 1)
            emit_oproj(b, 1)
            fill_drain()
            stage.pop(b, None)

    nc.compile()
    return nc


def _w_sbuf_layout(w_t):
    """[D, 128] weight-transpose slice -> SBUF layout [128, dc*128+o]."""
    return np.ascontiguousarray(
        w_t.reshape(DC, P, P).transpose(1, 0, 2).reshape(P, D))


def _tile_x(xT, dtype):
    """[D, B*S] -> gt-major tiled [NT*DC*P, TBLK] (rows: (gt*DC+dc)*P)."""
    t = xT.reshape(DC, P, NT, TBLK).transpose(2, 0, 1, 3)
    return np.ascontiguousarray(t.reshape(NT * DC * P, TBLK)).astype(dtype)


def _prepare_in_maps(inputs):
    bf = ml_dtypes.bfloat16
    xs = {}
    for name, t in (("queries", "q"), ("keys", "k"), ("values", "v")):
        x = np.asarray(inputs[name], dtype=np.float32)  # [B,S,D,2]
        flat = x.reshape(B * S, D, 2)
        xs[t + "r"] = _tile_x(flat[:, :, 0].T, bf)
        xs[t + "i"] = _tile_x(flat[:, :, 1].T, bf)

    scale = np.float32(1.0 / np.sqrt(DH))
    in_maps = []
    for c in range(NCORES):
        rows = slice(P * c, P * (c + 1))
        m = {}
        for t in ("q", "k", "v"):
            for part in ("r", "i"):
                m[f"x{t}_{part}"] = xs[t + part]
        packs = {}
        for t, wr_name, wi_name in (("q", "wq_r", "wq_i"),
                                    ("k", "wk_r", "wk_i"),
                                    ("v", "wv_r", "wv_i")):
            s = scale if t == "q" else np.float32(1.0)
            wr = np.asarray(inputs[wr_name], dtype=np.float32)[rows] * s
            wi = np.asarray(inputs[wi_name], dtype=np.float32)[rows] * s
            for h in range(HPC):
                hr = slice(DH * h, DH * (h + 1))
                if t == "q":
                    wa = np.concatenate([wr[hr].T, wi[hr].T], axis=1)
                    wb = np.concatenate([-wi[hr].T, wr[hr].T], axis=1)
                elif t == "k":
                    wa = np.concatenate([wr[hr].T, -wi[hr].T], axis=1)
                    wb = np.concatenate([-wi[hr].T, -wr[hr].T], axis=1)
                else:
                    wa = np.concatenate([wr[hr].T, wi[hr].T], axis=1)
                    wb = np.concatenate([-wi[hr].T, wr[hr].T], axis=1)
                packs[f"{t}a{h}"] = _w_sbuf_layout(wa)
                packs[f"{t}b{h}"] = _w_sbuf_layout(wb)
        wo_r = np.asarray(inputs["wo_r"], dtype=np.float32)[:, rows]  # [D,128]
        wo_i = np.asarray(inputs["wo_i"], dtype=np.float32)[:, rows]
        packs["wo_r"] = np.ascontiguousarray(wo_r.T)  # [128 d, 1024 m]
        packs["wo_i"] = np.ascontiguousarray(wo_i.T)
        packs["wo_in"] = np.ascontiguousarray(-wo_i.T)
        m["w_qpack"] = np.concatenate(
            [packs["q" + sfx] for sfx in QPACKS], axis=1).astype(bf)
        m["w_kvpack"] = np.concatenate(
            [packs[key] for key in KVPACKS], axis=1).astype(bf)
        m["ident"] = np.eye(P, dtype=bf)
        m["onesin"] = np.ones((P, P), dtype=bf)
        in_maps.append(m)
    return in_maps


LAST_RESULT = None


def _run(inputs, trace=False):
    global LAST_RESULT
    from concourse.bass_utils import run_bass_kernel_spmd
    if "nc" not in _CACHE:
        _CACHE["nc"] = _build()
    nc = _CACHE["nc"]
    in_maps = _prepare_in_maps(inputs)
    if trace:
        os.environ.pop("BASS_NEVER_TRACE", None)
    else:
        os.environ["BASS_NEVER_TRACE"] = "1"
    res = run_bass_kernel_spmd(nc, in_maps, core_ids=list(range(NCORES)),
                               trace=trace)
    LAST_RESULT = res
    acc_r = np.zeros((NT * DC * P, TBLK), np.float32)
    acc_i = np.zeros((NT * DC * P, TBLK), np.float32)
    for c in range(NCORES):
        acc_r += res.results[c]["po_r"].astype(np.float32)
        acc_i += res.results[c]["po_i"].astype(np.float32)

    def untile(po):
        # [NT*DC*P, TBLK] rows (gt*DC+mc)*P -> [D, B*S] -> [B,S,D]
        t = po.reshape(NT, DC, P, TBLK).transpose(1, 2, 0, 3)
        return np.ascontiguousarray(t.reshape(D, B * S)).T.reshape(B, S, D)

    out = np.empty((B, S, D, 2), np.float32)
    out[..., 0] = untile(acc_r)
    out[..., 1] = untile(acc_i)
    return out


def kernel(**inputs):
    return _run(inputs, trace=False)
